# revision 38
# baseline (speedup 1.0000x reference)
"""CapsuleTransformConv on 8 Trainium2 NeuronCores.

Problem:  x [4,16,16,32,16] f32, matrix [288,16,512] f32.
          im2col (K=3, VALID) -> tile [4,14,14,288,16]
          votes  = einsum('bhwna,nac->bhwnc', tile, matrix)
          out    = votes.reshape(4,14,14,288,32,16)

Sharding: tensor-parallel over the filter*atom output axis (512 -> 64 per
core).  Every core reads the full x (2 MB) and its 64-wide slice of the
weights; writes its [784, 288, 64] slice of the output (~58 MB, the
dominant HBM traffic).

Per-core kernel:
  - x is loaded once into SBUF as 8 slabs of [128 rows, 512 (c,a)].
  - PE transposes produce xT [(c_in_octet, atom)=128 partitions,
    4 octets x 1024 (b,h,w)]; the 9 im2col taps are then just windowed
    (strided) access patterns over xT's free dim -- x is read from HBM
    exactly once.
  - Weights for 8 consecutive capsules (one c-octet of one tap) are laid
    out block-diagonally in a [128, 512] tile so one K=128 matmul computes
    8 independent [pos,16]@[16,64] capsule matmuls: out[pos, gc*64+f].
    The block-diagonal tile is built on-chip from a single compact 1.2 MB
    weight DMA (memset + 8 strided copies).
  - Main loop: 7 position windows (112 = 4b x 2i x 14j) x 9 taps; each
    iteration runs 4 matmuls (c-octets) into one 4-bank PSUM tile,
    one PSUM->SBUF copy (alternating Vector/Scalar engines), and one
    ~918 KB DMA to HBM (2 KB contiguous runs).
  - Matmuls run in float32r (TF32-class PE mode, 4x the fp32 rate);
    accumulation is fp32 in PSUM.
"""

import numpy as np

B, H, W, C, A = 4, 16, 16, 32, 16
KS = 3
OH = OW = 14
NCAP = KS * KS * C          # 288 capsules
FTOT = 512                  # filter*atom
NCORES = 8
FPC = FTOT // NCORES        # 64 output features per core
POS = B * OH * OW           # 784 output positions
NG = NCAP // 8              # 36 groups of 8 capsules = (tap, c-octet)

_NC_CACHE = {}
MM_MODE = "f32r"  # "f32" (exact, 4 cyc/row) or "f32r" (TF32-class, 1 cyc/row)


def _build_nc(mm_f32r=True):
    import concourse.bass as bass  # noqa: F401
    import concourse.mybir as mybir
    import concourse.tile as tile
    from concourse import bacc, masks

    f32 = mybir.dt.float32
    mmdt = mybir.dt.float32r if mm_f32r else mybir.dt.float32

    nc = bacc.Bacc(None, target_bir_lowering=False)
    x_d = nc.declare_dram_parameter("x", [B, H, W, C, A], f32, isOutput=False)
    m_d = nc.declare_dram_parameter("mat", [NCAP, A, FPC], f32, isOutput=False)
    # Tap-major output layout: out[kk, pos, 32*64].  Each inner-loop DMA then
    # writes one fully contiguous ~0.7-0.9 MB block (vs 8 KB runs strided by
    # 72 KB in pos-major layout); the host transposes kk back into n.
    o_d = nc.declare_dram_parameter("out", [KS * KS, POS, 32 * FPC], f32,
                                    isOutput=True)

    x2d = x_d.rearrange("b h w c a -> (b h w) (c a)")   # [1024, 512]

    with tile.TileContext(nc) as tc:
        with (
            tc.tile_pool(name="const", bufs=1) as constp,
            tc.tile_pool(name="big", bufs=1) as bigp,
            tc.tile_pool(name="stage", bufs=3) as stagep,
            tc.tile_pool(name="tapp", bufs=2) as tapp,
            tc.tile_pool(name="psum", bufs=2, space="PSUM") as psump,
        ):
            ident = constp.tile([128, 128], f32, tag="ident")
            masks.make_identity(nc, ident[:])

            # ---- x: HBM -> SBUF once, as 8 row-slab tiles of [128, 512] ----
            # (separate tiles so each transpose depends only on its slab)
            x_sbs = [
                bigp.tile([128, 512], f32, tag=f"x_sb{s}", name=f"x_sb{s}")
                for s in range(8)
            ]
            for s in range(8):
                nc.sync.dma_start(x_sbs[s][:], x2d[s * 128:(s + 1) * 128, :])

            # ---- weights: block-diagonal wpack, built per-tap ----
            # wpack_c[(gc,a), oct*512 + gc*64 + f] = matrix[(c*4+oct)*8+gc, a, f]
            # else 0.  FP32r matmul inputs must be produced by a rounding
            # instruction (never by DMA), so paint DMAs land in transient f32
            # tiles and a full-partition engine copy rounds each chunk.
            # One chunk per tap kk so kk=0 matmuls start without waiting for
            # the whole weight build.  The two transient tiles are memset
            # once: every chunk paints the same diagonal positions, so the
            # off-diagonal zeros stay clean across reuse.
            msrc = m_d.rearrange("(g gc) a f -> gc a g f", gc=8)
            # One serially-reused paint buffer covering 4 taps (16 groups);
            # every round paints the same diagonal positions, so the memset
            # zeros stay clean across reuse.  3 paint rounds x 8 DMAs.
            wtmp = bigp.tile([128, 16 * 512], f32, tag="wtmp")
            # Split memset so tap-0's quarter is clean almost immediately.
            nc.gpsimd.memset(wtmp[:, 0:2048], 0.0)
            nc.gpsimd.memset(wtmp[:, 2048:], 0.0)
            wtv = wtmp[:].rearrange("p (g v) -> p g v", g=16)
            wpacks = []
            for rnd, ntap in ((0, 1), (1, 4), (2, 4)):
                g0 = (0, 4, 20)[rnd]  # first group of this round
                ng = ntap * 4
                for gc in range(8):
                    # Round 0 rides the sync ring (interleaves with x loads,
                    # fast path to the first matmul); later rounds go on the
                    # scalar ring, which is idle until outputs begin.
                    eng = nc.sync if rnd == 0 else nc.scalar
                    eng.dma_start(
                        wtv[gc * 16:(gc + 1) * 16, 0:ng,
                            gc * FPC:(gc + 1) * FPC],
                        msrc[gc, :, g0: g0 + ng, :],
                    )
                for t in range(ntap):
                    kk_of = g0 // 4 + t
                    wp = bigp.tile(
                        [128, 4 * 512], mmdt,
                        tag=f"wpack{kk_of}", name=f"wpack{kk_of}",
                    )
                    nc.vector.tensor_copy(
                        wp[:], wtmp[:, t * 2048:(t + 1) * 2048]
                    )
                    wpacks.append(wp)

            # ---- xT: PE-transpose x into 4 per-octet tiles [(dc,a), (b,h,w)]
            # Separate tiles so each octet's im2col cast can start as soon as
            # its own 8 transposes land.
            xts = [
                bigp.tile([128, 1024], f32, tag=f"xt{o}", name=f"xt{o}")
                for o in range(4)
            ]
            for oct in range(4):
                for s in range(8):
                    tr = psump.tile([128, 128], f32, tag="mm")
                    nc.tensor.transpose(
                        tr[:],
                        x_sbs[s][:, oct * 128:(oct + 1) * 128],
                        ident[:],
                    )
                    dst = xts[oct][:, s * 128:(s + 1) * 128]
                    if s % 2 == 0:
                        nc.vector.tensor_copy(dst, tr[:])
                    else:
                        nc.scalar.copy(dst, tr[:])

            xtvs = [
                t[:].rearrange("p (b h w) -> p b h w", b=B, h=H) for t in xts
            ]

            # ---- main loop: 9 taps (outer) x per-batch pos windows ----
            # The matmul stationary operand must be a single flat free dim
            # (walrus constraint), so per tap we compact the im2col gather
            # into tap[(dc,a), oct*784 + (b,i,j)] with GPSIMD copies.
            it = 0
            for kk in range(9):
                ki, kj = kk // 3, kk % 3
                tap = tapp.tile([128, 4 * POS], mmdt, tag="tap")
                for oct in range(4):
                    dst = tap[:, oct * POS:(oct + 1) * POS].rearrange(
                        "p (b i j) -> p b i j", b=B, i=OH
                    )
                    src = xtvs[oct][:, :, ki: ki + OH, kj: kj + OW]
                    if kk == 0:
                        # First tap on DVE/ACT (idle at startup) to cut the
                        # latency to the first matmul; rest on idle GPSIMD.
                        if oct % 2 == 0:
                            nc.vector.tensor_copy(dst, src)
                        else:
                            nc.scalar.copy(dst, src)
                    else:
                        nc.gpsimd.tensor_copy(dst, src)
                for b in range(B):
                    for i0, ni in ((0, 8), (8, 6)):
                        m = ni * OW  # 112 or 84 output positions
                        ps = psump.tile([128, 2048], f32, tag="mm")
                        for oct in range(4):
                            off = oct * POS + b * (OH * OW) + i0 * OW
                            nc.tensor.matmul(
                                ps[0:m, oct * 512:(oct + 1) * 512],
                                tap[:, off: off + m],
                                wpacks[kk][:, oct * 512:(oct + 1) * 512],
                                start=True,
                                stop=True,
                            )
                        st = stagep.tile([128, 2048], f32, tag="st")
                        # Split the PSUM->SBUF copy by banks so DVE and ACT
                        # run in parallel (different PSUM banks); DVE gets 3
                        # banks, ACT 1 (balances ACT's DMA-issue load).
                        nc.vector.tensor_copy(st[0:m, 0:1536], ps[0:m, 0:1536])
                        nc.scalar.copy(st[0:m, 1536:2048], ps[0:m, 1536:2048])
                        # Row-split the output DMA across both HWDGE rings so
                        # the write stream stays continuous.
                        q0 = b * (OH * OW) + i0 * OW
                        mh = 56
                        nc.sync.dma_start(
                            o_d[kk, q0: q0 + mh, :], st[0:mh, :]
                        )
                        nc.scalar.dma_start(
                            o_d[kk, q0 + mh: q0 + m, :], st[mh:m, :]
                        )
                        it += 1

    nc.compile()
    return nc


def _get_nc():
    key = MM_MODE
    if key not in _NC_CACHE:
        _NC_CACHE[key] = _build_nc(mm_f32r=(MM_MODE == "f32r"))
    return _NC_CACHE[key]


def kernel(x, matrix):
    from concourse.bass_utils import run_bass_kernel_spmd

    x = np.ascontiguousarray(x, dtype=np.float32)
    matrix = np.ascontiguousarray(matrix, dtype=np.float32)
    nc = _get_nc()
    in_maps = [
        {
            "x": x,
            "mat": np.ascontiguousarray(matrix[:, :, c * FPC:(c + 1) * FPC]),
        }
        for c in range(NCORES)
    ]
    r = run_bass_kernel_spmd(nc, in_maps, list(range(NCORES)))
    # parts[c]: [9, 784, 2048] tap-major -> [784, kk, 32, core, 64] -> full
    arr = np.stack([r.results[c]["out"] for c in range(NCORES)])
    arr = arr.reshape(NCORES, KS * KS, POS, 32, FPC)
    arr = arr.transpose(2, 1, 3, 0, 4)               # [pos, kk, 32, core, f]
    full = arr.reshape(POS, NCAP, FTOT)
    return np.ascontiguousarray(
        full.reshape(B, OH, OW, NCAP, 32, 16).astype(np.float32)
    )


# revision 39
# speedup vs baseline: 1.1703x; 1.1703x over previous
"""CapsuleTransformConv on 8 Trainium2 NeuronCores.

Problem:  x [4,16,16,32,16] f32, matrix [288,16,512] f32.
          im2col (K=3, VALID) -> tile [4,14,14,288,16]
          votes  = einsum('bhwna,nac->bhwnc', tile, matrix)
          out    = votes.reshape(4,14,14,288,32,16)

Sharding: tensor-parallel over the filter*atom output axis (512 -> 64 per
core).  Every core reads the full x (2 MB) and its 64-wide slice of the
weights; writes its [784, 288, 64] slice of the output (~58 MB, the
dominant HBM traffic).

Per-core kernel:
  - x is loaded once into SBUF as 8 slabs of [128 rows, 512 (c,a)].
  - PE transposes produce xT [(c_in_octet, atom)=128 partitions,
    4 octets x 1024 (b,h,w)]; the 9 im2col taps are then just windowed
    (strided) access patterns over xT's free dim -- x is read from HBM
    exactly once.
  - Weights for 8 consecutive capsules (one c-octet of one tap) are laid
    out block-diagonally in a [128, 512] tile so one K=128 matmul computes
    8 independent [pos,16]@[16,64] capsule matmuls: out[pos, gc*64+f].
    The block-diagonal tile is built on-chip from a single compact 1.2 MB
    weight DMA (memset + 8 strided copies).
  - Main loop: 7 position windows (112 = 4b x 2i x 14j) x 9 taps; each
    iteration runs 4 matmuls (c-octets) into one 4-bank PSUM tile,
    one PSUM->SBUF copy (alternating Vector/Scalar engines), and one
    ~918 KB DMA to HBM (2 KB contiguous runs).
  - Matmuls run in float32r (TF32-class PE mode, 4x the fp32 rate);
    accumulation is fp32 in PSUM.
"""

import numpy as np

B, H, W, C, A = 4, 16, 16, 32, 16
KS = 3
OH = OW = 14
NCAP = KS * KS * C          # 288 capsules
FTOT = 512                  # filter*atom
NCORES = 8
FPC = FTOT // NCORES        # 64 output features per core
POS = B * OH * OW           # 784 output positions
NG = NCAP // 8              # 36 groups of 8 capsules = (tap, c-octet)

_NC_CACHE = {}
MM_MODE = "f32r"  # "f32" (exact, 4 cyc/row) or "f32r" (TF32-class, 1 cyc/row)


def _build_nc(mm_f32r=True):
    import concourse.bass as bass  # noqa: F401
    import concourse.mybir as mybir
    import concourse.tile as tile
    from concourse import bacc, masks

    f32 = mybir.dt.float32
    mmdt = mybir.dt.float32r if mm_f32r else mybir.dt.float32

    nc = bacc.Bacc(None, target_bir_lowering=False)
    x_d = nc.declare_dram_parameter("x", [B, H, W, C, A], f32, isOutput=False)
    m_d = nc.declare_dram_parameter("mat", [NCAP, A, FPC], f32, isOutput=False)
    # Tap-major output layout: out[kk, pos, 32*64].  Each inner-loop DMA then
    # writes one fully contiguous ~0.7-0.9 MB block (vs 8 KB runs strided by
    # 72 KB in pos-major layout); the host transposes kk back into n.
    o_d = nc.declare_dram_parameter("out", [KS * KS, POS, 32 * FPC], f32,
                                    isOutput=True)

    x2d = x_d.rearrange("b h w c a -> (b h w) (c a)")   # [1024, 512]

    with tile.TileContext(nc) as tc:
        with (
            tc.tile_pool(name="const", bufs=1) as constp,
            tc.tile_pool(name="big", bufs=1) as bigp,
            tc.tile_pool(name="stage", bufs=3) as stagep,
            tc.tile_pool(name="tapp", bufs=2) as tapp,
            tc.tile_pool(name="psum", bufs=2, space="PSUM") as psump,
        ):
            ident = constp.tile([128, 128], f32, tag="ident")
            masks.make_identity(nc, ident[:])

            # ---- x: HBM -> SBUF once, as 8 row-slab tiles of [128, 512] ----
            # (separate tiles so each transpose depends only on its slab)
            x_sbs = [
                bigp.tile([128, 512], f32, tag=f"x_sb{s}", name=f"x_sb{s}")
                for s in range(8)
            ]
            for s in range(8):
                nc.sync.dma_start(x_sbs[s][:], x2d[s * 128:(s + 1) * 128, :])

            # ---- weights: block-diagonal wpack, built per-tap ----
            # wpack_c[(gc,a), oct*512 + gc*64 + f] = matrix[(c*4+oct)*8+gc, a, f]
            # else 0.  FP32r matmul inputs must be produced by a rounding
            # instruction (never by DMA), so paint DMAs land in transient f32
            # tiles and a full-partition engine copy rounds each chunk.
            # One chunk per tap kk so kk=0 matmuls start without waiting for
            # the whole weight build.  The two transient tiles are memset
            # once: every chunk paints the same diagonal positions, so the
            # off-diagonal zeros stay clean across reuse.
            msrc = m_d.rearrange("(g gc) a f -> gc a g f", gc=8)
            # One serially-reused paint buffer covering 4 taps (16 groups);
            # every round paints the same diagonal positions, so the memset
            # zeros stay clean across reuse.  3 paint rounds x 8 DMAs.
            wtmp = bigp.tile([128, 16 * 512], f32, tag="wtmp")
            # Split memset so tap-0's quarter is clean almost immediately.
            nc.gpsimd.memset(wtmp[:, 0:2048], 0.0)
            nc.gpsimd.memset(wtmp[:, 2048:], 0.0)
            wtv = wtmp[:].rearrange("p (g v) -> p g v", g=16)
            wpacks = []
            for rnd, ntap in ((0, 1), (1, 4), (2, 4)):
                g0 = (0, 4, 20)[rnd]  # first group of this round
                ng = ntap * 4
                for gc in range(8):
                    # Round 0 rides the sync ring (interleaves with x loads,
                    # fast path to the first matmul); later rounds go on the
                    # scalar ring, which is idle until outputs begin.
                    eng = nc.sync if rnd == 0 else nc.scalar
                    eng.dma_start(
                        wtv[gc * 16:(gc + 1) * 16, 0:ng,
                            gc * FPC:(gc + 1) * FPC],
                        msrc[gc, :, g0: g0 + ng, :],
                    )
                for t in range(ntap):
                    kk_of = g0 // 4 + t
                    wp = bigp.tile(
                        [128, 4 * 512], mmdt,
                        tag=f"wpack{kk_of}", name=f"wpack{kk_of}",
                    )
                    nc.vector.tensor_copy(
                        wp[:], wtmp[:, t * 2048:(t + 1) * 2048]
                    )
                    wpacks.append(wp)

            # ---- xT: PE-transpose x into 4 per-octet tiles [(dc,a), (b,h,w)]
            # Separate tiles so each octet's im2col cast can start as soon as
            # its own 8 transposes land.
            xts = [
                bigp.tile([128, 1024], f32, tag=f"xt{o}", name=f"xt{o}")
                for o in range(4)
            ]
            for oct in range(4):
                for s in range(8):
                    tr = psump.tile([128, 128], f32, tag="mm")
                    nc.tensor.transpose(
                        tr[:],
                        x_sbs[s][:, oct * 128:(oct + 1) * 128],
                        ident[:],
                    )
                    dst = xts[oct][:, s * 128:(s + 1) * 128]
                    if s % 2 == 0:
                        nc.vector.tensor_copy(dst, tr[:])
                    else:
                        nc.scalar.copy(dst, tr[:])

            xtvs = [
                t[:].rearrange("p (b h w) -> p b h w", b=B, h=H) for t in xts
            ]

            # ---- main loop: 9 taps (outer) x per-batch pos windows ----
            # The matmul stationary operand must be a single flat free dim
            # (walrus constraint), so per tap we compact the im2col gather
            # into tap[(dc,a), oct*784 + (b,i,j)] with GPSIMD copies.
            it = 0
            for kk in range(9):
                ki, kj = kk // 3, kk % 3
                tap = tapp.tile([128, 4 * POS], mmdt, tag="tap")
                for oct in range(4):
                    dst = tap[:, oct * POS:(oct + 1) * POS].rearrange(
                        "p (b i j) -> p b i j", b=B, i=OH
                    )
                    src = xtvs[oct][:, :, ki: ki + OH, kj: kj + OW]
                    if kk == 0:
                        # First tap on DVE/ACT (idle at startup) to cut the
                        # latency to the first matmul; rest on idle GPSIMD.
                        if oct % 2 == 0:
                            nc.vector.tensor_copy(dst, src)
                        else:
                            nc.scalar.copy(dst, src)
                    else:
                        nc.gpsimd.tensor_copy(dst, src)
                for b in range(B):
                    for i0, ni in ((0, 8), (8, 6)):
                        m = ni * OW  # 112 or 84 output positions
                        ps = psump.tile([128, 2048], f32, tag="mm")
                        for oct in range(4):
                            off = oct * POS + b * (OH * OW) + i0 * OW
                            nc.tensor.matmul(
                                ps[0:m, oct * 512:(oct + 1) * 512],
                                tap[:, off: off + m],
                                wpacks[kk][:, oct * 512:(oct + 1) * 512],
                                start=True,
                                stop=True,
                            )
                        st = stagep.tile([128, 2048], f32, tag="st")
                        # Split the PSUM->SBUF copy by bank pairs so DVE and
                        # ACT run in parallel (different PSUM banks).
                        nc.vector.tensor_copy(st[0:m, 0:1024], ps[0:m, 0:1024])
                        nc.scalar.copy(st[0:m, 1024:2048], ps[0:m, 1024:2048])
                        # Alternate the two HWDGE rings (SP / ACT) so output
                        # DMAs pipeline across both.
                        dma_eng = nc.sync if it % 2 == 0 else nc.scalar
                        q0 = b * (OH * OW) + i0 * OW
                        dma_eng.dma_start(
                            o_d[kk, q0: q0 + m, :],
                            st[0:m, :],
                        )
                        it += 1

    nc.compile()
    return nc


def _get_nc():
    key = MM_MODE
    if key not in _NC_CACHE:
        _NC_CACHE[key] = _build_nc(mm_f32r=(MM_MODE == "f32r"))
    return _NC_CACHE[key]


def kernel(x, matrix):
    from concourse.bass_utils import run_bass_kernel_spmd

    x = np.ascontiguousarray(x, dtype=np.float32)
    matrix = np.ascontiguousarray(matrix, dtype=np.float32)
    nc = _get_nc()
    in_maps = [
        {
            "x": x,
            "mat": np.ascontiguousarray(matrix[:, :, c * FPC:(c + 1) * FPC]),
        }
        for c in range(NCORES)
    ]
    r = run_bass_kernel_spmd(nc, in_maps, list(range(NCORES)))
    # parts[c]: [9, 784, 2048] tap-major -> [784, kk, 32, core, 64] -> full
    arr = np.stack([r.results[c]["out"] for c in range(NCORES)])
    arr = arr.reshape(NCORES, KS * KS, POS, 32, FPC)
    arr = arr.transpose(2, 1, 3, 0, 4)               # [pos, kk, 32, core, f]
    full = arr.reshape(POS, NCAP, FTOT)
    return np.ascontiguousarray(
        full.reshape(B, OH, OW, NCAP, 32, 16).astype(np.float32)
    )


# revision 40
# speedup vs baseline: 1.1730x; 1.0023x over previous
"""CapsuleTransformConv on 8 Trainium2 NeuronCores.

Problem:  x [4,16,16,32,16] f32, matrix [288,16,512] f32.
          im2col (K=3, VALID) -> tile [4,14,14,288,16]
          votes  = einsum('bhwna,nac->bhwnc', tile, matrix)
          out    = votes.reshape(4,14,14,288,32,16)

Sharding: tensor-parallel over the filter*atom output axis (512 -> 64 per
core).  Every core reads the full x (2 MB) and its 64-wide slice of the
weights; writes its [784, 288, 64] slice of the output (~58 MB, the
dominant HBM traffic).

Per-core kernel:
  - x is loaded once into SBUF as 8 slabs of [128 rows, 512 (c,a)].
  - PE transposes produce xT [(c_in_octet, atom)=128 partitions,
    4 octets x 1024 (b,h,w)]; the 9 im2col taps are then just windowed
    (strided) access patterns over xT's free dim -- x is read from HBM
    exactly once.
  - Weights for 8 consecutive capsules (one c-octet of one tap) are laid
    out block-diagonally in a [128, 512] tile so one K=128 matmul computes
    8 independent [pos,16]@[16,64] capsule matmuls: out[pos, gc*64+f].
    The block-diagonal tile is built on-chip from a single compact 1.2 MB
    weight DMA (memset + 8 strided copies).
  - Main loop: 7 position windows (112 = 4b x 2i x 14j) x 9 taps; each
    iteration runs 4 matmuls (c-octets) into one 4-bank PSUM tile,
    one PSUM->SBUF copy (alternating Vector/Scalar engines), and one
    ~918 KB DMA to HBM (2 KB contiguous runs).
  - Matmuls run in float32r (TF32-class PE mode, 4x the fp32 rate);
    accumulation is fp32 in PSUM.
"""

import numpy as np

B, H, W, C, A = 4, 16, 16, 32, 16
KS = 3
OH = OW = 14
NCAP = KS * KS * C          # 288 capsules
FTOT = 512                  # filter*atom
NCORES = 8
FPC = FTOT // NCORES        # 64 output features per core
POS = B * OH * OW           # 784 output positions
NG = NCAP // 8              # 36 groups of 8 capsules = (tap, c-octet)

_NC_CACHE = {}
MM_MODE = "f32r"  # "f32" (exact, 4 cyc/row) or "f32r" (TF32-class, 1 cyc/row)


def _build_nc(mm_f32r=True):
    import concourse.bass as bass  # noqa: F401
    import concourse.mybir as mybir
    import concourse.tile as tile
    from concourse import bacc, masks

    f32 = mybir.dt.float32
    mmdt = mybir.dt.float32r if mm_f32r else mybir.dt.float32

    nc = bacc.Bacc(None, target_bir_lowering=False)
    x_d = nc.declare_dram_parameter("x", [B, H, W, C, A], f32, isOutput=False)
    m_d = nc.declare_dram_parameter("mat", [NCAP, A, FPC], f32, isOutput=False)
    # Tap-major output layout: out[kk, pos, 32*64].  Each inner-loop DMA then
    # writes one fully contiguous ~0.7-0.9 MB block (vs 8 KB runs strided by
    # 72 KB in pos-major layout); the host transposes kk back into n.
    o_d = nc.declare_dram_parameter("out", [KS * KS, POS, 32 * FPC], f32,
                                    isOutput=True)

    x2d = x_d.rearrange("b h w c a -> (b h w) (c a)")   # [1024, 512]

    with tile.TileContext(nc) as tc:
        with (
            tc.tile_pool(name="const", bufs=1) as constp,
            tc.tile_pool(name="big", bufs=1) as bigp,
            tc.tile_pool(name="stage", bufs=3) as stagep,
            tc.tile_pool(name="tapp", bufs=2) as tapp,
            tc.tile_pool(name="psum", bufs=2, space="PSUM") as psump,
        ):
            ident = constp.tile([128, 128], f32, tag="ident")
            masks.make_identity(nc, ident[:])

            # ---- x: HBM -> SBUF once, as 8 row-slab tiles of [128, 512] ----
            # (separate tiles so each transpose depends only on its slab)
            x_sbs = [
                bigp.tile([128, 512], f32, tag=f"x_sb{s}", name=f"x_sb{s}")
                for s in range(8)
            ]
            for s in range(8):
                nc.sync.dma_start(x_sbs[s][:], x2d[s * 128:(s + 1) * 128, :])

            # ---- weights: block-diagonal wpack, built per-tap ----
            # wpack_c[(gc,a), oct*512 + gc*64 + f] = matrix[(c*4+oct)*8+gc, a, f]
            # else 0.  FP32r matmul inputs must be produced by a rounding
            # instruction (never by DMA), so paint DMAs land in transient f32
            # tiles and a full-partition engine copy rounds each chunk.
            # One chunk per tap kk so kk=0 matmuls start without waiting for
            # the whole weight build.  The two transient tiles are memset
            # once: every chunk paints the same diagonal positions, so the
            # off-diagonal zeros stay clean across reuse.
            msrc = m_d.rearrange("(g gc) a f -> gc a g f", gc=8)
            # One serially-reused paint buffer covering 4 taps (16 groups);
            # every round paints the same diagonal positions, so the memset
            # zeros stay clean across reuse.  3 paint rounds x 8 DMAs.
            wtmp = bigp.tile([128, 16 * 512], f32, tag="wtmp")
            # Split memset so tap-0's quarter is clean almost immediately.
            nc.gpsimd.memset(wtmp[:, 0:2048], 0.0)
            nc.gpsimd.memset(wtmp[:, 2048:], 0.0)
            wtv = wtmp[:].rearrange("p (g v) -> p g v", g=16)
            wpacks = []
            for rnd, ntap in ((0, 1), (1, 4), (2, 4)):
                g0 = (0, 4, 20)[rnd]  # first group of this round
                ng = ntap * 4
                for gc in range(8):
                    # Round 0 rides the sync ring (interleaves with x loads,
                    # fast path to the first matmul); later rounds go on the
                    # scalar ring, which is idle until outputs begin.
                    eng = nc.sync if rnd == 0 else nc.scalar
                    eng.dma_start(
                        wtv[gc * 16:(gc + 1) * 16, 0:ng,
                            gc * FPC:(gc + 1) * FPC],
                        msrc[gc, :, g0: g0 + ng, :],
                    )
                for t in range(ntap):
                    kk_of = g0 // 4 + t
                    wp = bigp.tile(
                        [128, 4 * 512], mmdt,
                        tag=f"wpack{kk_of}", name=f"wpack{kk_of}",
                    )
                    nc.vector.tensor_copy(
                        wp[:], wtmp[:, t * 2048:(t + 1) * 2048]
                    )
                    wpacks.append(wp)

            # ---- xT: PE-transpose x into 4 per-octet tiles [(dc,a), (b,h,w)]
            # Separate tiles so each octet's im2col cast can start as soon as
            # its own 8 transposes land.
            xts = [
                bigp.tile([128, 1024], f32, tag=f"xt{o}", name=f"xt{o}")
                for o in range(4)
            ]
            for oct in range(4):
                for s in range(8):
                    tr = psump.tile([128, 128], f32, tag="mm")
                    nc.tensor.transpose(
                        tr[:],
                        x_sbs[s][:, oct * 128:(oct + 1) * 128],
                        ident[:],
                    )
                    dst = xts[oct][:, s * 128:(s + 1) * 128]
                    if s % 2 == 0:
                        nc.vector.tensor_copy(dst, tr[:])
                    else:
                        nc.scalar.copy(dst, tr[:])

            xtvs = [
                t[:].rearrange("p (b h w) -> p b h w", b=B, h=H) for t in xts
            ]

            # ---- main loop: 9 taps (outer) x per-batch pos windows ----
            # The matmul stationary operand must be a single flat free dim
            # (walrus constraint), so per tap we compact the im2col gather
            # into tap[(dc,a), oct*784 + (b,i,j)] with GPSIMD copies.
            it = 0
            for kk in range(9):
                ki, kj = kk // 3, kk % 3
                tap = tapp.tile([128, 4 * POS], mmdt, tag="tap")
                for oct in range(4):
                    dst = tap[:, oct * POS:(oct + 1) * POS].rearrange(
                        "p (b i j) -> p b i j", b=B, i=OH
                    )
                    src = xtvs[oct][:, :, ki: ki + OH, kj: kj + OW]
                    if kk == 0:
                        # First tap per-batch on DVE/ACT (idle at startup):
                        # batch b's cast only needs x slabs 2b..2b+1, so the
                        # first matmul starts as soon as the first slabs
                        # transpose.  Later taps prefetch on idle GPSIMD.
                        for bb in range(B):
                            if (oct + bb) % 2 == 0:
                                nc.vector.tensor_copy(
                                    dst[:, bb], src[:, bb]
                                )
                            else:
                                nc.scalar.copy(dst[:, bb], src[:, bb])
                    else:
                        nc.gpsimd.tensor_copy(dst, src)
                for b in range(B):
                    for i0, ni in ((0, 8), (8, 6)):
                        m = ni * OW  # 112 or 84 output positions
                        ps = psump.tile([128, 2048], f32, tag="mm")
                        for oct in range(4):
                            off = oct * POS + b * (OH * OW) + i0 * OW
                            nc.tensor.matmul(
                                ps[0:m, oct * 512:(oct + 1) * 512],
                                tap[:, off: off + m],
                                wpacks[kk][:, oct * 512:(oct + 1) * 512],
                                start=True,
                                stop=True,
                            )
                        st = stagep.tile([128, 2048], f32, tag="st")
                        # Split the PSUM->SBUF copy by bank pairs so DVE and
                        # ACT run in parallel (different PSUM banks).
                        nc.vector.tensor_copy(st[0:m, 0:1024], ps[0:m, 0:1024])
                        nc.scalar.copy(st[0:m, 1024:2048], ps[0:m, 1024:2048])
                        # Alternate the two HWDGE rings (SP / ACT) so output
                        # DMAs pipeline across both.
                        dma_eng = nc.sync if it % 2 == 0 else nc.scalar
                        q0 = b * (OH * OW) + i0 * OW
                        dma_eng.dma_start(
                            o_d[kk, q0: q0 + m, :],
                            st[0:m, :],
                        )
                        it += 1

    nc.compile()
    return nc


def _get_nc():
    key = MM_MODE
    if key not in _NC_CACHE:
        _NC_CACHE[key] = _build_nc(mm_f32r=(MM_MODE == "f32r"))
    return _NC_CACHE[key]


def kernel(x, matrix):
    from concourse.bass_utils import run_bass_kernel_spmd

    x = np.ascontiguousarray(x, dtype=np.float32)
    matrix = np.ascontiguousarray(matrix, dtype=np.float32)
    nc = _get_nc()
    in_maps = [
        {
            "x": x,
            "mat": np.ascontiguousarray(matrix[:, :, c * FPC:(c + 1) * FPC]),
        }
        for c in range(NCORES)
    ]
    r = run_bass_kernel_spmd(nc, in_maps, list(range(NCORES)))
    # parts[c]: [9, 784, 2048] tap-major -> [784, kk, 32, core, 64] -> full
    arr = np.stack([r.results[c]["out"] for c in range(NCORES)])
    arr = arr.reshape(NCORES, KS * KS, POS, 32, FPC)
    arr = arr.transpose(2, 1, 3, 0, 4)               # [pos, kk, 32, core, f]
    full = arr.reshape(POS, NCAP, FTOT)
    return np.ascontiguousarray(
        full.reshape(B, OH, OW, NCAP, 32, 16).astype(np.float32)
    )
